# revision 1
# baseline (speedup 1.0000x reference)
"""Multi-head attention (B=4, S=2048, D=1024, H=16) on 8 NeuronCores.

Sharding: core c handles batch b = c//2 and head-group g = c%2 (8 heads each).
W_q/W_k/W_v are column-split per head group; W_o row-split; each core computes
a partial output for its batch which the host sums (row-parallel reduction).

Device layout strategy (per core):
  - inputs host-pretransposed: qt/kt/vt = X[b].T  [D, S]  (d on partitions)
  - q^T, k^T computed in [o, s] layout (o = head*64+dk on partitions)
  - v computed in natural [s, o] layout with a ones column per head (M=65
    matmuls emit softmax denominators for free)
  - per head pair: scores^T [sk, (2, sq512)] into one 2-bank PSUM tile,
    single wide exp (amortizes ACT's per-op overhead), attn@V accumulates
    num^T [65, (2, sq512)] into one 2-bank PSUM tile
  - normalize: reciprocal of the d rows, DMA partition-remap, gpsimd
    broadcast, one DVE multiply per head (odd head routed via DMA for the
    cross-partition write)
  - output projection interleaved per sq block; host adds partials + bo

All matmuls run in float32r (fp32 storage, ~2e-4 rel err, 1 cyc/row).
"""

import numpy as np

import concourse.bass as bass
import concourse.tile as tile
from concourse import bacc, mybir
from concourse.bass_utils import run_bass_kernel_spmd

FP = mybir.dt.float32
FR = mybir.dt.float32r
AF = mybir.ActivationFunctionType

B, S, D = 4, 2048, 1024
H, DK = 16, 64
HPC = 8          # heads per core
OC = HPC * DK    # 512 output cols per core
N_CORES = 8

ND = D // 128    # 8 d-tiles
NS = S // 128    # 16 s-tiles
NSB = S // 512   # 4 s-blocks
NO = OC // 128   # 4 o-tiles per core

_PROG_CACHE = {}


def build_program(repeats: int = 1, debug_dump: bool = False, phase: str = "full"):
    nc = bacc.Bacc("TRN2", target_bir_lowering=False, debug=False,
                   num_devices=N_CORES)

    qt = nc.dram_tensor("qt", [D, S], FR, kind="ExternalInput").ap()
    kt = nc.dram_tensor("kt", [D, S], FR, kind="ExternalInput").ap()
    vt = nc.dram_tensor("vt", [D, S], FR, kind="ExternalInput").ap()
    wqt = nc.dram_tensor("wqt", [D, OC], FR, kind="ExternalInput").ap()
    wkt = nc.dram_tensor("wkt", [D, OC], FR, kind="ExternalInput").ap()
    wvt = nc.dram_tensor("wvt", [D, OC], FR, kind="ExternalInput").ap()
    wot = nc.dram_tensor("wot", [OC, D], FR, kind="ExternalInput").ap()
    bq = nc.dram_tensor("bq", [OC], FP, kind="ExternalInput").ap()
    bk = nc.dram_tensor("bk", [OC], FP, kind="ExternalInput").ap()
    bv = nc.dram_tensor("bv", [OC], FP, kind="ExternalInput").ap()
    onec = nc.dram_tensor("onec", [128, HPC], FR, kind="ExternalInput").ap()
    y = nc.dram_tensor("y", [S, D], FP, kind="ExternalOutput").ap()
    if debug_dump:
        dbg_qt = nc.dram_tensor("dbg_qt", [OC, S], FP, kind="ExternalOutput").ap()
        dbg_kt = nc.dram_tensor("dbg_kt", [OC, S], FP, kind="ExternalOutput").ap()
        dbg_v = nc.dram_tensor("dbg_v", [S, HPC * 65], FP, kind="ExternalOutput").ap()
        dbg_ot = nc.dram_tensor("dbg_ot", [OC, S], FP, kind="ExternalOutput").ap()

    with tile.TileContext(nc) as tc:
        def body(_iv=None):
            with tc.tile_pool(name="pers_o", bufs=1) as pers_o, \
                 tc.tile_pool(name="consts", bufs=1) as consts, \
                 tc.tile_pool(name="psum", bufs=1, space="PSUM") as psum, \
                 tc.tile_pool(name="fstage", bufs=1) as fstage, \
                 tc.tile_pool(name="yout", bufs=4) as ypool:
                oT = [pers_o.tile([128, S], FR, tag=f"oT{i}", name=f"oT{i}")
                      for i in range(NO)]
                ones_sb = consts.tile([128, HPC], FR, tag="ones")
                nc.sync.dma_start(out=ones_sb[:], in_=onec[:])
                bq_t = consts.tile([128, NO], FP, tag="bq")
                nc.sync.dma_start(out=bq_t[:], in_=bq.rearrange("(ot oi) -> oi ot", oi=128))
                bk_t = consts.tile([128, NO], FP, tag="bk")
                nc.sync.dma_start(out=bk_t[:], in_=bk.rearrange("(ot oi) -> oi ot", oi=128))
                bv_bc = consts.tile([128, OC], FP, tag="bv")
                nc.sync.dma_start(out=bv_bc[:], in_=bv.partition_broadcast(128))
                wo_t = fstage.tile([128, NO, D], FR, tag="wo")
                nc.sync.dma_start(
                    out=wo_t[:], in_=wot.rearrange("(oo oi) yd -> oi oo yd", oi=128))

                with tc.tile_pool(name="pers_qkv", bufs=1) as pers:
                    qT = [pers.tile([128, S], FR, tag=f"qT{i}", name=f"qT{i}")
                          for i in range(NO)]
                    kT = [pers.tile([128, S], FR, tag=f"kT{i}", name=f"kT{i}")
                          for i in range(NO)]
                    vsb = [pers.tile([128, HPC * 65], FR, tag=f"v{i}", name=f"v{i}")
                           for i in range(NS)]

                    # ---------------- projections ----------------
                    with tc.tile_pool(name="stage", bufs=2) as stage, \
                         tc.tile_pool(name="wstage", bufs=1) as wstage:
                        for xt_d, wt_d, bias_t, outT in (
                            (qt, wqt, bq_t, qT),
                            (kt, wkt, bk_t, kT),
                        ):
                            wt = wstage.tile([128, ND, OC], FR, tag="w", name="wt")
                            nc.sync.dma_start(
                                out=wt[:],
                                in_=wt_d.rearrange("(dd di) o -> di dd o", di=128))
                            for sb in range(NSB):
                                xs = stage.tile([128, ND, 512], FR, tag="xs", name="xs")
                                nc.sync.dma_start(
                                    out=xs[:],
                                    in_=xt_d.rearrange("(dd di) s -> di dd s", di=128)
                                        [:, :, sb * 512:(sb + 1) * 512])
                                for ot in range(NO):
                                    ps = psum.tile([128, 512], FP, tag="sc", bufs=2,
                                                   name="ps")
                                    for dt in range(ND):
                                        nc.tensor.matmul(
                                            ps[:],
                                            wt[:, dt, ot * 128:(ot + 1) * 128],
                                            xs[:, dt, :],
                                            start=(dt == 0), stop=(dt == ND - 1))
                                    nc.vector.tensor_scalar_add(
                                        outT[ot][:, sb * 512:(sb + 1) * 512],
                                        ps[:], bias_t[:, ot:ot + 1])

                        wt = wstage.tile([128, ND, OC], FR, tag="w", name="wt")
                        nc.sync.dma_start(
                            out=wt[:],
                            in_=wvt.rearrange("(dd di) o -> di dd o", di=128))
                        for st in range(NS):
                            xs = stage.tile([128, ND, 128], FR, tag="xs", name="xs")
                            nc.sync.dma_start(
                                out=xs[:],
                                in_=vt.rearrange("(dd di) s -> di dd s", di=128)
                                    [:, :, st * 128:(st + 1) * 128])
                            ps = psum.tile([128, 512], FP, tag="sc", bufs=2, name="ps")
                            for dt in range(ND):
                                nc.tensor.matmul(
                                    ps[:], xs[:, dt, :], wt[:, dt, :],
                                    start=(dt == 0), stop=(dt == ND - 1))
                            vv = vsb[st].rearrange("p (h c) -> p h c", c=65)
                            nc.vector.tensor_add(
                                vv[:, :, 0:64],
                                ps.rearrange("p (h c) -> p h c", c=64),
                                bv_bc.rearrange("p (h c) -> p h c", c=64))
                            nc.vector.tensor_copy(vv[:, :, 64:65], ones_sb.unsqueeze(2))

                    if debug_dump:
                        for i in range(NO):
                            nc.sync.dma_start(out=dbg_qt[i * 128:(i + 1) * 128, :],
                                              in_=qT[i][:].bitcast(FP))
                            nc.sync.dma_start(out=dbg_kt[i * 128:(i + 1) * 128, :],
                                              in_=kT[i][:].bitcast(FP))
                        for i in range(NS):
                            nc.sync.dma_start(out=dbg_v[i * 128:(i + 1) * 128, :],
                                              in_=vsb[i][:].bitcast(FP))
                    if phase == "proj":
                        yf = y.rearrange("s d2 -> (s d2)")
                        n = 0
                        for i in range(NO):
                            for t in (qT[i], kT[i]):
                                nc.sync.dma_start(
                                    out=yf[n * 65536:(n + 1) * 65536]
                                        .rearrange("(p f) -> p f", p=128),
                                    in_=t[:, 0:512].bitcast(FP))
                                n += 1
                        for i in range(NS):
                            nc.sync.dma_start(
                                out=yf[n * 65536 + i * 66560:
                                       n * 65536 + (i + 1) * 66560]
                                    .rearrange("(p f) -> p f", p=128),
                                in_=vsb[i][:].bitcast(FP))
                        return

                    # ------------- attention (+ interleaved out-proj) -------------
                    with tc.tile_pool(name="et", bufs=4) as epool, \
                         tc.tile_pool(name="nrm", bufs=2) as npool:
                        for sq in range(NSB):
                            sqs = slice(sq * 512, (sq + 1) * 512)
                            for p in range(NO):     # heads 2p, 2p+1
                                nump = psum.tile([65, 1024], FP, tag="num", bufs=2,
                                                 name="nump")
                                for sk in range(NS):
                                    scs = psum.tile([128, 1024], FP, tag="sc", bufs=2,
                                                    name="scs")
                                    for e in range(2):
                                        nc.tensor.matmul(
                                            scs[:, e * 512:(e + 1) * 512],
                                            kT[p][e * 64:(e + 1) * 64,
                                                  sk * 128:(sk + 1) * 128],
                                            qT[p][e * 64:(e + 1) * 64, sqs],
                                            start=True, stop=True)
                                    et = epool.tile([128, 1024], FR, tag="et", name="et")
                                    nc.scalar.activation(et[:], scs[:], AF.Exp,
                                                         scale=0.125)
                                    for e in range(2):
                                        h = 2 * p + e
                                        nc.tensor.matmul(
                                            nump[:, e * 512:(e + 1) * 512],
                                            vsb[sk][:, h * 65:(h + 1) * 65],
                                            et[:, e * 512:(e + 1) * 512],
                                            start=(sk == 0), stop=(sk == NS - 1),
                                            skip_group_check=True)
                                rec = npool.tile([65, 1024], FP, tag="rec", name="rec")
                                nc.vector.reciprocal(rec[64:65, :], nump[64:65, :])
                                rec0 = npool.tile([1, 1024], FP, tag="rec0", name="rec0")
                                nc.sync.dma_start(out=rec0[:], in_=rec[64:65, :])
                                bc = npool.tile([64, 1024], FP, tag="bc", name="bc")
                                nc.gpsimd.partition_broadcast(bc[:], rec0[:])
                                nc.vector.tensor_mul(
                                    oT[p][0:64, sqs], nump[0:64, 0:512], bc[:, 0:512])
                                tmp = npool.tile([64, 512], FR, tag="tmp", name="tmp")
                                nc.vector.tensor_mul(tmp[:], nump[0:64, 512:1024],
                                                     bc[:, 512:1024])
                                nc.sync.dma_start(out=oT[p][64:128, sqs], in_=tmp[:])

                            if phase == "attn":
                                continue
                            # ---- output projection for this sq block ----
                            for stl in range(NSB):
                                st = sq * NSB + stl
                                sts = slice(st * 128, (st + 1) * 128)
                                pss = [psum.tile([128, 512], FP, tag="num", bufs=2,
                                                 name=f"yps{i}") for i in range(2)]
                                for o4 in range(NO):
                                    for yb in range(2):
                                        nc.tensor.matmul(
                                            pss[yb][:],
                                            oT[o4][:, sts],
                                            wo_t[:, o4, yb * 512:(yb + 1) * 512],
                                            start=(o4 == 0), stop=(o4 == NO - 1),
                                            skip_group_check=True)
                                for yb in range(2):
                                    yt = ypool.tile([128, 512], FP, tag="yt", name="yt")
                                    nc.vector.tensor_copy(yt[:], pss[yb][:])
                                    nc.sync.dma_start(
                                        out=y[sts, yb * 512:(yb + 1) * 512], in_=yt[:])

                        if debug_dump:
                            for i in range(NO):
                                nc.sync.dma_start(out=dbg_ot[i * 128:(i + 1) * 128, :],
                                                  in_=oT[i][:].bitcast(FP))
                        if phase == "attn":
                            for i in range(NO):
                                nc.sync.dma_start(out=y[i * 128:(i + 1) * 128, :],
                                                  in_=oT[i][:, 0:1024].bitcast(FP))

        if repeats == 1:
            body()
        else:
            with tc.For_i(0, repeats, 1) as iv:
                body(iv)

    nc.compile()
    return nc


def _get_prog(repeats: int = 1):
    if repeats not in _PROG_CACHE:
        _PROG_CACHE[repeats] = build_program(repeats)
    return _PROG_CACHE[repeats]


def make_in_maps(Q, K, V, Wq, bq, Wk, bk, Wv, bv, Wo, bo):
    Q, K, V = (np.asarray(x, dtype=np.float32) for x in (Q, K, V))
    Wq, Wk, Wv, Wo = (np.asarray(x, dtype=np.float32) for x in (Wq, Wk, Wv, Wo))
    bq, bk, bv = (np.asarray(x, dtype=np.float32) for x in (bq, bk, bv))

    qt_b = [np.ascontiguousarray(Q[b].T) for b in range(B)]
    kt_b = [np.ascontiguousarray(K[b].T) for b in range(B)]
    vt_b = [np.ascontiguousarray(V[b].T) for b in range(B)]
    wqt_g = [np.ascontiguousarray(Wq.T[:, g * OC:(g + 1) * OC]) for g in range(2)]
    wkt_g = [np.ascontiguousarray(Wk.T[:, g * OC:(g + 1) * OC]) for g in range(2)]
    wvt_g = [np.ascontiguousarray(Wv.T[:, g * OC:(g + 1) * OC]) for g in range(2)]
    wot_g = [np.ascontiguousarray(Wo.T[g * OC:(g + 1) * OC, :]) for g in range(2)]
    onec = np.ones((128, HPC), dtype=np.float32)

    in_maps = []
    for c in range(N_CORES):
        b, g = c // 2, c % 2
        in_maps.append({
            "qt": qt_b[b], "kt": kt_b[b], "vt": vt_b[b],
            "wqt": wqt_g[g], "wkt": wkt_g[g], "wvt": wvt_g[g],
            "wot": wot_g[g],
            "bq": np.ascontiguousarray(bq[g * OC:(g + 1) * OC]),
            "bk": np.ascontiguousarray(bk[g * OC:(g + 1) * OC]),
            "bv": np.ascontiguousarray(bv[g * OC:(g + 1) * OC]),
            "onec": onec,
        })
    return in_maps


def gather_output(results, bo):
    bo = np.asarray(bo, dtype=np.float32)
    Y = np.empty((B, S, D), dtype=np.float32)
    for b in range(B):
        Y[b] = results[2 * b]["y"] + results[2 * b + 1]["y"] + bo
    return Y


def kernel(Q, K, V, Wq, bq, Wk, bk, Wv, bv, Wo, bo):
    nc = _get_prog()
    in_maps = make_in_maps(Q, K, V, Wq, bq, Wk, bk, Wv, bv, Wo, bo)
    res = run_bass_kernel_spmd(nc, in_maps, list(range(N_CORES)))
    return gather_output(res.results, bo)

